# revision 22
# baseline (speedup 1.0000x reference)
# Trainium2 Bass kernel for nn_MeshUnpool (gnn_message_passing).
#
# Reference semantics (per mesh b):
#   out = (features[b] @ unroll_mat[b][mask_rows]) / occ
# The 0/1 unroll matrix is ~0.07% dense, so the unpool is a dense
# [128, e] @ [e, ncol] matmul per mesh after dropping all-zero rows/columns
# (one mesh per core, pure data parallel).
#
# Key structural tricks (v6):
#   - DEGREE-1 COLUMN FACTORING: ~28% of kept output columns have exactly
#     ONE source row, i.e. only one of the 23 contraction chunks touches
#     them.  Columns are laid out as [multi-degree block | 23 per-chunk
#     64-wide degree-1 slots], so chunk k's matmul pass sweeps the multi
#     block plus ONLY ITS OWN slot: 23*(2624+64) = 61.8K PE cycles instead
#     of 23*3648 = 83.9K (-26%).  Slot width 64 divides the 512-column PSUM
#     bank, so slot matmuls never cross a bank.  ncol_dev = 2624 + 23*64 =
#     4096 = exactly the 8 PSUM banks.
#   - The 8 warmup matmuls that ramp the PE out of its cold p-state ALSO
#     zero the 8 PSUM banks (zeros tile, start=True); every real matmul
#     accumulates with start=False (+ skip_group_check since groups span
#     mixed sub-bank regions).
#   - W ships BIT-PACKED two columns per byte (col j as 0x38 in bits 3-5,
#     col j+2048 as 0x07 in bits 0-2).  One fused DVE op per half expands a
#     chunk to fp8: (x & 0x3838) and (x & 0x0707) << 3 both yield the
#     fp8e4m3 pattern of 1.0 exactly (DVE runs these at 2x: ~530ns each).
#   - each chunk's 256B fp16 stationary rides IN FRONT of its packed W row
#     (read through a bitcast AP): one DMA per chunk delivers both.
#   - the W stream round-robins over THREE DMA queues (SP + Act HWDGE +
#     gpsimd SWDGE) to ride out per-queue bandwidth variance.
#   - occurrences division + scatter back to [128, 4096] happen on host;
#     out ships fp16; redundant LDWEIGHTS stripped post-compile.

import numpy as np
import ml_dtypes

B, NF, E, U = 8, 128, 3072, 4096
NCORES = 8
AB = 256   # stationary bytes per partition packed ahead of each W chunk row
S1 = 64    # degree-1 slot width per chunk (divides the 512-col PSUM bank)
PSUM_COLS = 4096

_compiled = {}


def _build_bass(kc, ncol, m_pad):
    """Per-core program: kc 128-row chunks; chunk k sweeps the multi-degree
    block [0, m_pad) plus its own degree-1 slot [m_pad+S1*k, +S1)."""
    import concourse.bass as bass
    import concourse.bacc as bacc
    import concourse.mybir as mybir
    import concourse.tile as tile

    nc = bacc.Bacc("TRN2", target_bir_lowering=False, debug=False)
    fp8 = mybir.dt.float8e4
    f16 = mybir.dt.float16
    f32 = mybir.dt.float32
    u16 = mybir.dt.uint16

    half = ncol // 2
    rowb = AB + half  # bytes per partition per chunk: [fp16 A | packed W]
    w = nc.dram_tensor("w", [128, kc, rowb], fp8, kind="ExternalInput").ap()
    out = nc.dram_tensor("out", [128, ncol], f16, kind="ExternalOutput").ap()

    # multi-degree block in 512-col slices (never cross a PSUM bank)
    mslices = []
    off = 0
    while off < m_pad:
        wd = min(512, m_pad - off)
        mslices.append((off, wd))
        off += wd

    def ranges_for(k):
        # (offset, width, is_slot) ranges chunk k contributes to
        return [(o, wd, False) for o, wd in mslices] + [(m_pad + S1 * k, S1, True)]

    # PSUM tiles of 1024 (2 banks each); ncol is a multiple of 1024 here
    ptiles = [(o, min(1024, ncol - o)) for o in range(0, ncol, 1024)]

    def locate(coff):
        return coff // 1024, coff % 1024

    with tile.TileContext(nc) as tc:
        with (
            tc.tile_pool(name="zpool", bufs=1) as zpool,
            tc.tile_pool(name="wpool", bufs=8) as wpool,
            tc.tile_pool(name="upool", bufs=4) as upool,
            tc.tile_pool(name="psum", bufs=1, space=bass.MemorySpace.PSUM) as ppool,
            tc.tile_pool(name="opool", bufs=4) as opool,
        ):
            z_t = zpool.tile([128, 512], fp8, tag="z")
            psums = [
                ppool.tile([128, wd], f32, tag=f"ps{i}", name=f"ps{i}")
                for i, (o, wd) in enumerate(ptiles)
            ]

            # Warmup doubles as PSUM zeroing: 8 bank-wide matmuls on a zeroed
            # tile (start=True) clear all accumulators while ramping the PE
            # out of its cold p-state; a few more keep it busy until the
            # first W chunk lands.  All real matmuls accumulate onto these.
            nc.vector.memset(z_t[:], 0)
            for bank in range(8):
                ti, lo = locate(bank * 512)
                nc.tensor.matmul(
                    psums[ti][:, lo : lo + 512], z_t[:, 0:128], z_t[:],
                    start=True, stop=False, skip_group_check=True,
                )
            for _ in range(4):
                nc.tensor.matmul(
                    psums[0][:, 0:512], z_t[:, 0:128], z_t[:],
                    start=False, stop=False, skip_group_check=True,
                )

            def mm(wu, coff, cw, stop):
                w_t, u_t = wu
                ti, lo = locate(coff)
                nc.tensor.matmul(
                    psums[ti][:, lo : lo + cw],
                    w_t[:, 0:AB].bitcast(f16),
                    u_t[:, coff : coff + cw],
                    start=False, stop=stop, skip_group_check=True,
                )

            def unpack(w_t):
                # expand packed chunk to fp8 0x00/0x38: one fused DVE op per
                # column half, operating on uint16 views (2x lane packing)
                u_t = upool.tile([128, ncol], fp8, tag="u")
                xin = w_t[:, AB:rowb].bitcast(u16)
                nc.vector.tensor_scalar(
                    u_t[:, 0:half].bitcast(u16), xin, 0x3838, 0,
                    mybir.AluOpType.bitwise_and, mybir.AluOpType.bypass,
                )
                nc.vector.tensor_scalar(
                    u_t[:, half:ncol].bitcast(u16), xin, 0x0707, 3,
                    mybir.AluOpType.bitwise_and, mybir.AluOpType.logical_shift_left,
                )
                return u_t

            for k in range(kc):
                w_t = wpool.tile([128, rowb], fp8, tag="w")
                if k == 0:
                    # prologue: first chunk split so the first unpack+matmuls
                    # wait on less data
                    c1 = AB + half // 2
                    nc.sync.dma_start(w_t[:, 0:c1], w[:, 0, 0:c1])
                    nc.sync.dma_start(w_t[:, c1:rowb], w[:, 0, c1:rowb])
                elif k % 3 == 1:
                    nc.scalar.dma_start(w_t[:], w[:, k, :])
                elif k % 3 == 2:
                    nc.gpsimd.dma_start(w_t[:], w[:, k, :])
                else:
                    nc.sync.dma_start(w_t[:], w[:, k, :])
                wu = (w_t, unpack(w_t))

                if k < kc - 2:
                    for coff, cw, is_slot in ranges_for(k):
                        mm(wu, coff, cw, stop=is_slot)
                elif k == kc - 2:
                    wu_prev = wu  # final two chunks run per-PSUM-tile below
                else:
                    # final two chunks: finish per PSUM tile, evict to fp16
                    # and store while the remaining tiles' matmuls drain;
                    # casts alternate DVE/Act so two evict chains overlap
                    for t, (toff, twd) in enumerate(ptiles):
                        for kk, wt in ((k - 1, wu_prev), (k, wu)):
                            for coff, cw, is_slot in ranges_for(kk):
                                if toff <= coff < toff + twd:
                                    mm(wt, coff, cw, stop=(is_slot or kk == k))
                        # evict in two 512-col halves on DVE and Act in
                        # PARALLEL so each tile's cast->store chain is short
                        o_t = opool.tile([128, 1024], f16, tag="o")
                        h = twd // 2
                        nc.vector.tensor_copy(o_t[:, 0:h], psums[t][:, 0:h])
                        nc.scalar.copy(o_t[:, h:twd], psums[t][:, h:twd])
                        q2 = nc.sync if t % 2 == 0 else nc.scalar
                        q2.dma_start(out[:, toff : toff + twd], o_t[:, 0:twd])

    nc.compile()
    _dedup_ldweights(nc)
    return nc


def _dedup_ldweights(nc):
    """Remove InstLdweights that reload the PE array with the exact weights it
    already holds (consecutive matmuls sharing one stationary operand).  The
    tile legalizer emits one LDWEIGHTS per matmul and neither it nor walrus
    dedups, so slice groups sharing a lhsT pay redundant ~100ns array loads
    each -- pure serial PE time.  Safe here because each stationary region is
    written once (per wpool slot generation) before its matmuls.  Any
    waits/updates on a removed LDW are transferred to the next PE inst."""
    import concourse.mybir as mybir

    for blk in nc.m.functions[0].blocks:
        insts = blk.instructions
        loaded = None
        pending = []  # sync infos of removed LDWs, to merge into next PE inst
        idx = 0
        while idx < len(insts):
            inst = insts[idx]
            if isinstance(inst, mybir.InstLdweights):
                key = (
                    str(inst.ins[0]),
                    str(inst.tile_position),
                    str(inst.perf_mode),
                    str(inst.is_transpose),
                )
                if loaded == key:
                    si = inst.sync_info
                    if si is not None and (si.on_wait or si.on_update):
                        pending.append(si)
                    del insts[idx]
                    continue
                loaded = key
            elif isinstance(inst, mybir.InstMatmult) and pending:
                si = inst.sync_info
                if si is None:
                    si = mybir.SyncInfo(on_wait=[], on_update=[])
                for p in pending:
                    si.on_wait = list(si.on_wait) + list(p.on_wait)
                    si.on_update = list(si.on_update) + list(p.on_update)
                inst.sync_info = si
                pending = []
            idx += 1
        assert not pending, "dangling sync from removed LDWEIGHTS"


def _get_compiled(*key):
    if key not in _compiled:
        _compiled[key] = _build_bass(*key)
    return _compiled[key]


def _prep_cores(features, unroll_mat, occurrences, dst_masks):
    """Host-side prep: mask-gather W rows, drop empty rows/cols, factor
    degree-1 columns into per-source-chunk slots, bit-pack W two columns per
    byte, pack the fp16 stationary in front of each chunk row."""
    per = []
    for b in range(B):
        wg = unroll_mat[b][dst_masks[b]]          # [E, U], entries 0/1
        keep = wg.any(axis=1)
        wk = wg[keep]
        fk = features[b][:, keep]                  # matching feature columns
        colidx0 = np.where(wk.any(axis=0))[0]
        wkk = wk[:, colidx0]
        per.append((fk, wkk, colidx0))

    rmax = max(fk.shape[1] for fk, _, _ in per)
    kc = (rmax + 127) // 128
    e = kc * 128

    # group degree-1 columns by source chunk, capped at S1 per chunk
    # (excess spills back into the multi block); geometry is shared SPMD-wide
    grouped = []
    for fk, wkk, colidx0 in per:
        deg = wkk.sum(axis=0)
        d1 = np.where(deg == 1)[0]
        multi = list(np.where(deg != 1)[0])
        srcs = wkk[:, d1].argmax(axis=0)
        slots = [[] for _ in range(kc)]
        for ci, s in zip(d1, srcs):
            ch = int(s) // 128
            if len(slots[ch]) < S1:
                slots[ch].append(ci)
            else:
                multi.append(ci)
        grouped.append((multi, slots))
    m_max = max(len(m) for m, _ in grouped)
    m_pad = (m_max + 63) // 64 * 64
    ncol = m_pad + kc * S1
    assert ncol <= PSUM_COLS, f"layout exceeds PSUM: {ncol}"
    half = ncol // 2
    rowb = AB + half

    in_maps, meta = [], []
    for (fk, wkk, colidx0), (multi, slots) in zip(per, grouped):
        r = fk.shape[1]
        at = np.zeros((e, 128), dtype=np.float32)  # A^T, zero-padded rows
        at[:r] = fk.T
        a3 = np.ascontiguousarray(
            at.astype(np.float16).reshape(kc, 128, 128).transpose(1, 0, 2)
        )
        # device column layout: [multi | slot 0 | slot 1 | ...]
        wnew = np.zeros((e, ncol), dtype=np.uint8)
        pos = np.empty(len(colidx0), dtype=np.int64)
        orig = np.empty(len(colidx0), dtype=np.int64)
        n = len(multi)
        wnew[:r, :n] = wkk[:, multi]
        pos[:n] = np.arange(n)
        orig[:n] = colidx0[multi]
        for k, sl in enumerate(slots):
            if sl:
                o = m_pad + S1 * k
                wnew[:r, o : o + len(sl)] = wkk[:, sl]
                pos[n : n + len(sl)] = o + np.arange(len(sl))
                orig[n : n + len(sl)] = colidx0[sl]
                n += len(sl)
        assert n == len(colidx0)
        # two columns per byte: col j -> bits 3-5 (0x38), col j+half -> 0-2
        wbits = wnew[:, :half] * np.uint8(0x38) | wnew[:, half:] * np.uint8(0x07)
        w3 = np.ascontiguousarray(wbits.reshape(kc, 128, half).transpose(1, 0, 2))
        packed = np.empty((128, kc, rowb), dtype=np.uint8)
        packed[:, :, :AB] = a3.view(np.uint8).reshape(128, kc, AB)
        packed[:, :, AB:] = w3
        in_maps.append({"w": packed.view(ml_dtypes.float8_e4m3)})
        meta.append((orig, pos))
    return kc, ncol, m_pad, in_maps, meta


def kernel(features, unroll_mat, occurrences, dst_masks):
    import concourse.bass_utils as bass_utils

    features = np.asarray(features, dtype=np.float32)
    unroll_mat = np.asarray(unroll_mat, dtype=np.float32)
    occurrences = np.asarray(occurrences, dtype=np.float32)
    dst_masks = np.asarray(dst_masks).astype(bool)

    kc, ncol, m_pad, in_maps, meta = _prep_cores(
        features, unroll_mat, occurrences, dst_masks
    )
    nc = _get_compiled(kc, ncol, m_pad)
    try:
        res = bass_utils.run_bass_kernel_spmd(
            nc, in_maps, core_ids=list(range(NCORES))
        )
    except Exception:
        # one retry for transient device hiccups (e.g. a wedged exec unit)
        res = bass_utils.run_bass_kernel_spmd(
            nc, in_maps, core_ids=list(range(NCORES))
        )
    occ = occurrences.reshape(B, U)
    full = np.zeros((B, NF, U), dtype=np.float32)
    for b in range(B):
        orig, pos = meta[b]
        dev = np.asarray(res.results[b]["out"])[:, pos].astype(np.float32)
        full[b][:, orig] = dev / occ[b, orig][None, :]
    return full


# revision 23
# speedup vs baseline: 1.0693x; 1.0693x over previous
# Trainium2 Bass kernel for nn_MeshUnpool (gnn_message_passing).
#
# Reference semantics (per mesh b):
#   out = (features[b] @ unroll_mat[b][mask_rows]) / occ
# The 0/1 unroll matrix is ~0.07% dense, so the unpool is a dense
# [128, e] @ [e, ncol] matmul per mesh after dropping all-zero rows/columns
# (one mesh per core, pure data parallel).
#
# Key structural tricks (v6):
#   - DEGREE-1 COLUMN FACTORING: ~28% of kept output columns have exactly
#     ONE source row, i.e. only one of the 23 contraction chunks touches
#     them.  Columns are laid out as [multi-degree block | 23 per-chunk
#     64-wide degree-1 slots], so chunk k's matmul pass sweeps the multi
#     block plus ONLY ITS OWN slot: 23*(2624+64) = 61.8K PE cycles instead
#     of 23*3648 = 83.9K (-26%).  Slot width 64 divides the 512-column PSUM
#     bank, so slot matmuls never cross a bank.  ncol_dev = 2624 + 23*64 =
#     4096 = exactly the 8 PSUM banks.
#   - The 8 warmup matmuls that ramp the PE out of its cold p-state ALSO
#     zero the 8 PSUM banks (zeros tile, start=True); every real matmul
#     accumulates with start=False (+ skip_group_check since groups span
#     mixed sub-bank regions).
#   - W ships BIT-PACKED two columns per byte (col j as 0x38 in bits 3-5,
#     col j+2048 as 0x07 in bits 0-2).  One fused DVE op per half expands a
#     chunk to fp8: (x & 0x3838) and (x & 0x0707) << 3 both yield the
#     fp8e4m3 pattern of 1.0 exactly (DVE runs these at 2x: ~530ns each).
#   - each chunk's 256B fp16 stationary rides IN FRONT of its packed W row
#     (read through a bitcast AP): one DMA per chunk delivers both.
#   - the W stream round-robins over THREE DMA queues (SP + Act HWDGE +
#     gpsimd SWDGE) to ride out per-queue bandwidth variance.
#   - occurrences division + scatter back to [128, 4096] happen on host;
#     out ships fp16; redundant LDWEIGHTS stripped post-compile.

import numpy as np
import ml_dtypes

B, NF, E, U = 8, 128, 3072, 4096
NCORES = 8
AB = 256   # stationary bytes per partition packed ahead of each W chunk row
S1 = 64    # degree-1 slot width per chunk (divides the 512-col PSUM bank)
PSUM_COLS = 4096

_compiled = {}


def _build_bass(kc, ncol, m_pad):
    """Per-core program: kc 128-row chunks; chunk k sweeps the multi-degree
    block [0, m_pad) plus its own degree-1 slot [m_pad+S1*k, +S1)."""
    import concourse.bass as bass
    import concourse.bacc as bacc
    import concourse.mybir as mybir
    import concourse.tile as tile

    nc = bacc.Bacc("TRN2", target_bir_lowering=False, debug=False)
    fp8 = mybir.dt.float8e4
    f16 = mybir.dt.float16
    f32 = mybir.dt.float32
    u16 = mybir.dt.uint16

    half = ncol // 2
    rowb = AB + half  # bytes per partition per chunk: [fp16 A | packed W]
    w = nc.dram_tensor("w", [128, kc, rowb], fp8, kind="ExternalInput").ap()
    out = nc.dram_tensor("out", [128, ncol], f16, kind="ExternalOutput").ap()

    # multi-degree block in 512-col slices (never cross a PSUM bank)
    mslices = []
    off = 0
    while off < m_pad:
        wd = min(512, m_pad - off)
        mslices.append((off, wd))
        off += wd

    def ranges_for(k):
        # (offset, width, is_slot) ranges chunk k contributes to
        return [(o, wd, False) for o, wd in mslices] + [(m_pad + S1 * k, S1, True)]

    # PSUM tiles of 1024 (2 banks each); ncol is a multiple of 1024 here
    ptiles = [(o, min(1024, ncol - o)) for o in range(0, ncol, 1024)]

    def locate(coff):
        return coff // 1024, coff % 1024

    with tile.TileContext(nc) as tc:
        with (
            tc.tile_pool(name="zpool", bufs=1) as zpool,
            tc.tile_pool(name="wpool", bufs=8) as wpool,
            tc.tile_pool(name="upool", bufs=4) as upool,
            tc.tile_pool(name="psum", bufs=1, space=bass.MemorySpace.PSUM) as ppool,
            tc.tile_pool(name="opool", bufs=4) as opool,
        ):
            z_t = zpool.tile([128, 512], fp8, tag="z")
            psums = [
                ppool.tile([128, wd], f32, tag=f"ps{i}", name=f"ps{i}")
                for i, (o, wd) in enumerate(ptiles)
            ]

            # Warmup doubles as PSUM zeroing: 8 bank-wide matmuls on a zeroed
            # tile (start=True) clear all accumulators while ramping the PE
            # out of its cold p-state; a few more keep it busy until the
            # first W chunk lands.  All real matmuls accumulate onto these.
            nc.vector.memset(z_t[:], 0)
            for bank in range(8):
                ti, lo = locate(bank * 512)
                nc.tensor.matmul(
                    psums[ti][:, lo : lo + 512], z_t[:, 0:128], z_t[:],
                    start=True, stop=False, skip_group_check=True,
                )
            for _ in range(4):
                nc.tensor.matmul(
                    psums[0][:, 0:512], z_t[:, 0:128], z_t[:],
                    start=False, stop=False, skip_group_check=True,
                )

            def mm(wu, coff, cw, stop):
                w_t, u_t = wu
                ti, lo = locate(coff)
                nc.tensor.matmul(
                    psums[ti][:, lo : lo + cw],
                    w_t[:, 0:AB].bitcast(f16),
                    u_t[:, coff : coff + cw],
                    start=False, stop=stop, skip_group_check=True,
                )

            def unpack(w_t):
                # expand packed chunk to fp8 0x00/0x38: one fused DVE op per
                # column half, operating on uint16 views (2x lane packing)
                u_t = upool.tile([128, ncol], fp8, tag="u")
                xin = w_t[:, AB:rowb].bitcast(u16)
                nc.vector.tensor_scalar(
                    u_t[:, 0:half].bitcast(u16), xin, 0x3838, 0,
                    mybir.AluOpType.bitwise_and, mybir.AluOpType.bypass,
                )
                nc.vector.tensor_scalar(
                    u_t[:, half:ncol].bitcast(u16), xin, 0x0707, 3,
                    mybir.AluOpType.bitwise_and, mybir.AluOpType.logical_shift_left,
                )
                return u_t

            for k in range(kc):
                w_t = wpool.tile([128, rowb], fp8, tag="w")
                if k == 0:
                    # prologue: first chunk split so the first unpack+matmuls
                    # wait on less data
                    c1 = AB + half // 2
                    nc.sync.dma_start(w_t[:, 0:c1], w[:, 0, 0:c1])
                    nc.sync.dma_start(w_t[:, c1:rowb], w[:, 0, c1:rowb])
                elif k % 3 == 1:
                    nc.scalar.dma_start(w_t[:], w[:, k, :])
                elif k % 3 == 2:
                    nc.gpsimd.dma_start(w_t[:], w[:, k, :])
                else:
                    nc.sync.dma_start(w_t[:], w[:, k, :])
                wu = (w_t, unpack(w_t))

                if k < kc - 2:
                    for coff, cw, is_slot in ranges_for(k):
                        mm(wu, coff, cw, stop=is_slot)
                elif k == kc - 2:
                    wu_prev = wu  # final two chunks run per-PSUM-tile below
                else:
                    # final two chunks: finish per PSUM tile, evict to fp16
                    # and store while the remaining tiles' matmuls drain;
                    # casts alternate DVE/Act so two evict chains overlap
                    for t, (toff, twd) in enumerate(ptiles):
                        for kk, wt in ((k - 1, wu_prev), (k, wu)):
                            for coff, cw, is_slot in ranges_for(kk):
                                if toff <= coff < toff + twd:
                                    mm(wt, coff, cw, stop=(is_slot or kk == k))
                        o_t = opool.tile([128, 1024], f16, tag="o")
                        if t % 2 == 0:
                            nc.vector.tensor_copy(o_t[:, 0:twd], psums[t][:])
                        else:
                            nc.scalar.copy(o_t[:, 0:twd], psums[t][:])
                        q2 = nc.sync if t % 2 == 0 else nc.scalar
                        q2.dma_start(out[:, toff : toff + twd], o_t[:, 0:twd])

    nc.compile()
    _dedup_ldweights(nc)
    return nc


def _dedup_ldweights(nc):
    """Remove InstLdweights that reload the PE array with the exact weights it
    already holds (consecutive matmuls sharing one stationary operand).  The
    tile legalizer emits one LDWEIGHTS per matmul and neither it nor walrus
    dedups, so slice groups sharing a lhsT pay redundant ~100ns array loads
    each -- pure serial PE time.  Safe here because each stationary region is
    written once (per wpool slot generation) before its matmuls.  Any
    waits/updates on a removed LDW are transferred to the next PE inst."""
    import concourse.mybir as mybir

    for blk in nc.m.functions[0].blocks:
        insts = blk.instructions
        loaded = None
        pending = []  # sync infos of removed LDWs, to merge into next PE inst
        idx = 0
        while idx < len(insts):
            inst = insts[idx]
            if isinstance(inst, mybir.InstLdweights):
                key = (
                    str(inst.ins[0]),
                    str(inst.tile_position),
                    str(inst.perf_mode),
                    str(inst.is_transpose),
                )
                if loaded == key:
                    si = inst.sync_info
                    if si is not None and (si.on_wait or si.on_update):
                        pending.append(si)
                    del insts[idx]
                    continue
                loaded = key
            elif isinstance(inst, mybir.InstMatmult) and pending:
                si = inst.sync_info
                if si is None:
                    si = mybir.SyncInfo(on_wait=[], on_update=[])
                for p in pending:
                    si.on_wait = list(si.on_wait) + list(p.on_wait)
                    si.on_update = list(si.on_update) + list(p.on_update)
                inst.sync_info = si
                pending = []
            idx += 1
        assert not pending, "dangling sync from removed LDWEIGHTS"


def _get_compiled(*key):
    if key not in _compiled:
        _compiled[key] = _build_bass(*key)
    return _compiled[key]


def _prep_cores(features, unroll_mat, occurrences, dst_masks):
    """Host-side prep: mask-gather W rows, drop empty rows/cols, factor
    degree-1 columns into per-source-chunk slots, bit-pack W two columns per
    byte, pack the fp16 stationary in front of each chunk row."""
    per = []
    for b in range(B):
        wg = unroll_mat[b][dst_masks[b]]          # [E, U], entries 0/1
        keep = wg.any(axis=1)
        wk = wg[keep]
        fk = features[b][:, keep]                  # matching feature columns
        colidx0 = np.where(wk.any(axis=0))[0]
        wkk = wk[:, colidx0]
        per.append((fk, wkk, colidx0))

    rmax = max(fk.shape[1] for fk, _, _ in per)
    kc = (rmax + 127) // 128
    e = kc * 128

    # group degree-1 columns by source chunk, capped at S1 per chunk
    # (excess spills back into the multi block); geometry is shared SPMD-wide
    grouped = []
    for fk, wkk, colidx0 in per:
        deg = wkk.sum(axis=0)
        d1 = np.where(deg == 1)[0]
        multi = list(np.where(deg != 1)[0])
        srcs = wkk[:, d1].argmax(axis=0)
        slots = [[] for _ in range(kc)]
        for ci, s in zip(d1, srcs):
            ch = int(s) // 128
            if len(slots[ch]) < S1:
                slots[ch].append(ci)
            else:
                multi.append(ci)
        grouped.append((multi, slots))
    m_max = max(len(m) for m, _ in grouped)
    m_pad = (m_max + 63) // 64 * 64
    ncol = m_pad + kc * S1
    assert ncol <= PSUM_COLS, f"layout exceeds PSUM: {ncol}"
    half = ncol // 2
    rowb = AB + half

    in_maps, meta = [], []
    for (fk, wkk, colidx0), (multi, slots) in zip(per, grouped):
        r = fk.shape[1]
        at = np.zeros((e, 128), dtype=np.float32)  # A^T, zero-padded rows
        at[:r] = fk.T
        a3 = np.ascontiguousarray(
            at.astype(np.float16).reshape(kc, 128, 128).transpose(1, 0, 2)
        )
        # device column layout: [multi | slot 0 | slot 1 | ...]
        wnew = np.zeros((e, ncol), dtype=np.uint8)
        pos = np.empty(len(colidx0), dtype=np.int64)
        orig = np.empty(len(colidx0), dtype=np.int64)
        n = len(multi)
        wnew[:r, :n] = wkk[:, multi]
        pos[:n] = np.arange(n)
        orig[:n] = colidx0[multi]
        for k, sl in enumerate(slots):
            if sl:
                o = m_pad + S1 * k
                wnew[:r, o : o + len(sl)] = wkk[:, sl]
                pos[n : n + len(sl)] = o + np.arange(len(sl))
                orig[n : n + len(sl)] = colidx0[sl]
                n += len(sl)
        assert n == len(colidx0)
        # two columns per byte: col j -> bits 3-5 (0x38), col j+half -> 0-2
        wbits = wnew[:, :half] * np.uint8(0x38) | wnew[:, half:] * np.uint8(0x07)
        w3 = np.ascontiguousarray(wbits.reshape(kc, 128, half).transpose(1, 0, 2))
        packed = np.empty((128, kc, rowb), dtype=np.uint8)
        packed[:, :, :AB] = a3.view(np.uint8).reshape(128, kc, AB)
        packed[:, :, AB:] = w3
        in_maps.append({"w": packed.view(ml_dtypes.float8_e4m3)})
        meta.append((orig, pos))
    return kc, ncol, m_pad, in_maps, meta


def kernel(features, unroll_mat, occurrences, dst_masks):
    import concourse.bass_utils as bass_utils

    features = np.asarray(features, dtype=np.float32)
    unroll_mat = np.asarray(unroll_mat, dtype=np.float32)
    occurrences = np.asarray(occurrences, dtype=np.float32)
    dst_masks = np.asarray(dst_masks).astype(bool)

    kc, ncol, m_pad, in_maps, meta = _prep_cores(
        features, unroll_mat, occurrences, dst_masks
    )
    nc = _get_compiled(kc, ncol, m_pad)
    try:
        res = bass_utils.run_bass_kernel_spmd(
            nc, in_maps, core_ids=list(range(NCORES))
        )
    except Exception:
        # one retry for transient device hiccups (e.g. a wedged exec unit)
        res = bass_utils.run_bass_kernel_spmd(
            nc, in_maps, core_ids=list(range(NCORES))
        )
    occ = occurrences.reshape(B, U)
    full = np.zeros((B, NF, U), dtype=np.float32)
    for b in range(B):
        orig, pos = meta[b]
        dev = np.asarray(res.results[b]["out"])[:, pos].astype(np.float32)
        full[b][:, orig] = dev / occ[b, orig][None, :]
    return full
